# revision 18
# baseline (speedup 1.0000x reference)
"""AFT transformer block on 8 Trainium2 NeuronCores.

Data-parallel over batch: each core runs the full block for 4 of the 32
sequences (the AFT attention mixes only within a sequence, so no
collectives are needed).  Host side folds the shared LayerNorm affine
params into the GEMM weights, precomputes exp(wbias).T, and casts all
weights to bf16; the device kernel computes bf16 matmuls with f32 PSUM
accumulation and f32 element-wise math.

ScalarE table-set discipline: two table loads total (exp for attention,
gelu for the FFN).  LayerNorm rstd is computed entirely on the vector
engine (bit-trick seed + Newton rsqrt), sigmoid uses e/(1+e) with
e = exp(Q), and divisions use the fast DVE reciprocal approximation.
"""

import numpy as np
import ml_dtypes

import concourse.bass as bass
import concourse.mybir as mybir
import concourse.tile as tile
from concourse import bacc
from concourse.bass_utils import run_bass_kernel_spmd
from concourse.masks import make_identity

F32 = mybir.dt.float32
U32 = mybir.dt.uint32
BF16 = mybir.dt.bfloat16
AF = mybir.ActivationFunctionType
ALU = mybir.AluOpType

B, T, D, FF = 32, 512, 1024, 4096
NCORES = 8
NB = B // NCORES          # sequences per core (4)
NT = NB * T               # tokens per core (2048)
KD = D // 128             # 8
KF = FF // 128            # 32
TT = NT // 128            # 16 token tiles per core
EPS = 1e-5


def build_nc(with_bias: bool):
    nc = bacc.Bacc("TRN2", target_bir_lowering=False, debug=False, num_devices=NCORES)

    x_ext = nc.dram_tensor("x", [NT, D], F32, kind="ExternalInput").ap()
    wq_ext = nc.dram_tensor("wq", [D, D], BF16, kind="ExternalInput").ap()
    wk_ext = nc.dram_tensor("wk", [D, D], BF16, kind="ExternalInput").ap()
    wv_ext = nc.dram_tensor("wv", [D, D], BF16, kind="ExternalInput").ap()
    wp_ext = nc.dram_tensor("wp", [D, D], BF16, kind="ExternalInput").ap()
    # w1 pre-tiled on host: w1f[f, p, k*128+c] = w1[k*128+p, f*128+c]
    w1f_ext = nc.dram_tensor("w1f", [KF, 128, D], BF16, kind="ExternalInput").ap()
    w2_ext = nc.dram_tensor("w2", [FF, D], BF16, kind="ExternalInput").ap()
    wbT_ext = nc.dram_tensor("wbT", [T, T], BF16, kind="ExternalInput").ap()
    # rows: 0=bk', 1=bv', 2=bp', 3=b2' (rank-1 bias matmul operands)
    brow_ext = nc.dram_tensor("brow", [4, D], BF16, kind="ExternalInput").ap()
    bqcol_ext = nc.dram_tensor("bqcol", [128, KD], F32, kind="ExternalInput").ap()
    b1col_ext = nc.dram_tensor("b1col", [128, KF], F32, kind="ExternalInput").ap()
    out_ext = nc.dram_tensor("out", [NT, D], F32, kind="ExternalOutput").ap()

    with tile.TileContext(nc) as tc:
        with (
            tc.tile_pool(name="const", bufs=1) as constp,
            tc.tile_pool(name="dstats", bufs=1) as dstats,
            tc.tile_pool(name="ps_t", bufs=2, space="PSUM") as ps_t,
            tc.tile_pool(name="x1dp", bufs=1, space="DRAM") as x1dp,
        ):
            ident = constp.tile([128, 128], BF16, tag="ident")
            make_identity(nc, ident)
            ones1 = constp.tile([1, 128], BF16, tag="ones1")
            nc.vector.memset(ones1, 1.0)
            wbT_sb = constp.tile([128, 4, T], BF16, tag="wbT")
            brows = []
            for r in range(4):
                br = constp.tile([1, D], BF16, tag=f"brow{r}", name=f"brow{r}")
                nc.sync.dma_start(out=br, in_=brow_ext[r:r + 1, :])
                brows.append(br)
            bqcol = constp.tile([128, KD], F32, tag="bqcol")
            nc.sync.dma_start(out=bqcol, in_=bqcol_ext[:, :])
            b1col = constp.tile([128, KF], F32, tag="b1col")
            nc.sync.dma_start(out=b1col, in_=b1col_ext[:, :])

            # per-token-tile LayerNorm stats, [128, tt] (one column per tile)
            mv1 = dstats.tile([128, TT, 2], F32, tag="mv1")
            rstd1 = dstats.tile([128, TT], F32, tag="rstd1")
            nmr1 = dstats.tile([128, TT], F32, tag="nmr1")
            mv2 = dstats.tile([128, TT, 2], F32, tag="mv2")
            rstd2 = dstats.tile([128, TT], F32, tag="rstd2")
            nmr2 = dstats.tile([128, TT], F32, tag="nmr2")
            nsc = dstats.tile([128, 4, 2], F32, tag="nsc")  # rsqrt Newton scratch

            def rsqrt_dve(rstd_t, nmr_t, mv_t, lo, hi):
                """rstd = 1/sqrt(var+eps), nmr = -mean*rstd — DVE only."""
                n = hi - lo
                y = rstd_t[:, lo:hi]
                ve = nsc[:, 0:n, 0]
                t = nsc[:, 0:n, 1]
                nc.vector.tensor_scalar_add(out=ve, in0=mv_t[:, lo:hi, 1], scalar1=EPS)
                nc.vector.tensor_scalar(out=y.bitcast(U32), in0=ve.bitcast(U32),
                                        scalar1=1, scalar2=None,
                                        op0=ALU.logical_shift_right)
                nc.vector.tensor_scalar(out=y.bitcast(U32), in0=y.bitcast(U32),
                                        scalar1=0x5f3759df, scalar2=-1,
                                        op0=ALU.subtract, op1=ALU.mult)
                for _ in range(3):
                    nc.vector.tensor_tensor(out=t, in0=ve, in1=y, op=ALU.mult)
                    nc.vector.tensor_tensor(out=t, in0=t, in1=y, op=ALU.mult)
                    nc.vector.tensor_scalar(out=t, in0=t, scalar1=-0.5, scalar2=1.5,
                                            op0=ALU.mult, op1=ALU.add)
                    nc.vector.tensor_tensor(out=y, in0=y, in1=t, op=ALU.mult)
                nc.vector.tensor_tensor(out=nmr_t[:, lo:hi], in0=mv_t[:, lo:hi, 0],
                                        in1=y, op=ALU.mult)
                nc.vector.tensor_scalar_mul(out=nmr_t[:, lo:hi], in0=nmr_t[:, lo:hi],
                                            scalar1=-1.0)

            # DRAM scratch for the post-attention residual stream x1
            x1d = [x1dp.tile([128, D], F32, tag=f"x1d{t}", name=f"x1d{t}") for t in range(TT)]

            # ---------------- attention sub-block ----------------
            with (
                tc.tile_pool(name="aw", bufs=1) as aw,
                tc.tile_pool(name="xbp", bufs=2) as xbp,
                tc.tile_pool(name="ps_mm", bufs=6, space="PSUM") as ps_mm,
            ):
                # LN1 stats prologue.  Batch 0's activation tiles go first in
                # the DMA queue; stats for tiles 0-3 are finalized early so
                # batch 0 can start while the rest of the stats stream in.
                xb0 = [xbp.tile([128, D], F32, tag=f"xb{i}", name=f"xb{i}") for i in range(4)]
                for i in range(4):
                    nc.sync.dma_start(out=xb0[i], in_=x_ext[i * 128:(i + 1) * 128, :])

                def ln_stats(xt, mv_t, t, stats_pool):
                    st = stats_pool.tile([128, 2, 6], F32, tag="bnst", name="st")
                    nc.vector.bn_stats(out=st[:, 0, :], in_=xt[:, 0:512])
                    nc.vector.bn_stats(out=st[:, 1, :], in_=xt[:, 512:1024])
                    nc.vector.bn_aggr(out=mv_t[:, t, :], in_=st)

                wq_sb = [aw.tile([128, D], BF16, tag=f"wq{k}", name=f"wq{k}") for k in range(KD)]
                wk_sb = [aw.tile([128, D], BF16, tag=f"wk{k}", name=f"wk{k}") for k in range(KD)]
                wv_sb = [aw.tile([128, D], BF16, tag=f"wv{k}", name=f"wv{k}") for k in range(KD)]
                wp_sb = [aw.tile([128, D], BF16, tag=f"wp{k}", name=f"wp{k}") for k in range(KD)]
                for k in range(KD):
                    sl = slice(k * 128, (k + 1) * 128)
                    nc.sync.dma_start(out=wq_sb[k], in_=wq_ext[sl, :])
                for k in range(KD):
                    sl = slice(k * 128, (k + 1) * 128)
                    nc.sync.dma_start(out=wk_sb[k], in_=wk_ext[sl, :])
                    nc.sync.dma_start(out=wv_sb[k], in_=wv_ext[sl, :])

                for s in range(4):
                    nc.sync.dma_start(out=wbT_sb[:, s, :], in_=wbT_ext[s * 128:(s + 1) * 128, :])
                for k in range(KD):
                    sl = slice(k * 128, (k + 1) * 128)
                    nc.sync.dma_start(out=wp_sb[k], in_=wp_ext[sl, :])

                with (
                    tc.tile_pool(name="ab", bufs=1) as ab,
                    tc.tile_pool(name="tmp", bufs=2) as tmp,
                    tc.tile_pool(name="sip", bufs=3) as sip,
                ):
                    # batch-0 stats and rstd straight away (DVE only)
                    for t in range(4):
                        ln_stats(xb0[t], mv1, t, sip)
                    rsqrt_dve(rstd1, nmr1, mv1, 0, 4)

                    xb = xb0
                    for b in range(NB):
                        h0T = [ab.tile([128, T], BF16, tag=f"h0T{k}", bufs=2, name=f"h0T{k}")
                               for k in range(KD)]
                        # prefetch next batch's activations early in the queue
                        if b + 1 < NB:
                            xb_next = [xbp.tile([128, D], F32, tag=f"xb{i}", name=f"xb{i}")
                                       for i in range(4)]
                            for i in range(4):
                                t = (b + 1) * 4 + i
                                nc.sync.dma_start(out=xb_next[i],
                                                  in_=x_ext[t * 128:(t + 1) * 128, :])
                        eK = [ab.tile([128, D], BF16, tag=f"eK{i}", bufs=1, name=f"eK{i}")
                              for i in range(4)]
                        EV = [ab.tile([128, D], BF16, tag=f"EV{i}", bufs=1, name=f"EV{i}")
                              for i in range(4)]
                        eQ = [ab.tile([128, T], BF16, tag=f"eQ{j}", bufs=1, name=f"eQ{j}")
                              for j in range(KD)]
                        e1 = [ab.tile([128, T], BF16, tag=f"e1{j}", bufs=1, name=f"e1{j}")
                              for j in range(KD)]
                        YT = [ab.tile([128, T], BF16, tag=f"YT{j}", bufs=1, name=f"YT{j}")
                              for j in range(KD)]
                        x1t = [ab.tile([128, D], F32, tag=f"x1t{i}", bufs=1, name=f"x1t{i}")
                               for i in range(4)]

                        # normalize (pre-folded LN) + transpose to feature-major
                        for i in range(4):
                            t = b * 4 + i
                            h0 = tmp.tile([128, D], BF16, tag="h0", bufs=3)
                            nc.scalar.activation(out=h0, in_=xb[i], func=AF.Identity,
                                                 bias=nmr1[:, t:t + 1], scale=rstd1[:, t:t + 1])
                            for k in range(KD):
                                tp = ps_t.tile([128, 128], BF16, tag="tp")
                                nc.tensor.transpose(tp, h0[:, k * 128:(k + 1) * 128], ident)
                                nc.vector.tensor_copy(out=h0T[k][:, i * 128:(i + 1) * 128],
                                                      in_=tp)

                        # next batch's LN1 stats early in the DVE stream so
                        # rstd1(b+1) is ready well before batch b+1 begins
                        if b + 1 < NB:
                            for i in range(4):
                                ln_stats(xb_next[i], mv1, (b + 1) * 4 + i, sip)
                            rsqrt_dve(rstd1, nmr1, mv1, (b + 1) * 4, (b + 1) * 4 + 4)

                        # Q -> e = exp(Q + bq)  (sigmoid deferred into Y)
                        for j in range(KD):
                            qps = ps_mm.tile([128, T], F32, tag="mm")
                            for k in range(KD):
                                nc.tensor.matmul(qps, lhsT=wq_sb[k][:, j * 128:(j + 1) * 128],
                                                 rhs=h0T[k], start=(k == 0), stop=(k == KD - 1))
                            nc.scalar.activation(out=eQ[j], in_=qps, func=AF.Exp,
                                                 bias=bqcol[:, j:j + 1])
                            nc.gpsimd.tensor_scalar_add(out=e1[j], in0=eQ[j], scalar1=1.0)

                        # K, V (token-major) -> exp(K), exp(K)*V
                        for i in range(4):
                            for h in range(2):
                                sl = slice(h * 512, (h + 1) * 512)
                                kps = ps_mm.tile([128, 512], F32, tag="mm")
                                for k in range(KD):
                                    nc.tensor.matmul(kps, lhsT=h0T[k][:, i * 128:(i + 1) * 128],
                                                     rhs=wk_sb[k][:, sl], start=(k == 0),
                                                     stop=(not with_bias and k == KD - 1))
                                if with_bias:
                                    nc.tensor.matmul(kps, lhsT=ones1, rhs=brows[0][:, sl],
                                                     start=False, stop=True)
                                nc.scalar.activation(out=eK[i][:, sl], in_=kps, func=AF.Exp)
                                vps = ps_mm.tile([128, 512], F32, tag="mm")
                                for k in range(KD):
                                    nc.tensor.matmul(vps, lhsT=h0T[k][:, i * 128:(i + 1) * 128],
                                                     rhs=wv_sb[k][:, sl], start=(k == 0),
                                                     stop=(not with_bias and k == KD - 1))
                                if with_bias:
                                    nc.tensor.matmul(vps, lhsT=ones1, rhs=brows[1][:, sl],
                                                     start=False, stop=True)
                                nc.vector.tensor_tensor(out=EV[i][:, sl], in0=eK[i][:, sl],
                                                        in1=vps, op=ALU.mult)

                        # positional-bias matmuls (feature-major) + Y epilogue
                        # Y = num * e * 1/((1+e) * den)
                        for j in range(KD):
                            jsl = slice(j * 128, (j + 1) * 128)
                            dps = ps_mm.tile([128, T], F32, tag="mm")
                            for s in range(4):
                                nc.tensor.matmul(dps, lhsT=eK[s][:, jsl], rhs=wbT_sb[:, s, :],
                                                 start=(s == 0), stop=(s == 3))
                            nps = ps_mm.tile([128, T], F32, tag="mm")
                            for s in range(4):
                                nc.tensor.matmul(nps, lhsT=EV[s][:, jsl], rhs=wbT_sb[:, s, :],
                                                 start=(s == 0), stop=(s == 3))
                            dd = tmp.tile([128, T], F32, tag="dd")
                            nc.vector.tensor_tensor(out=dd, in0=dps, in1=e1[j], op=ALU.mult)
                            rd = tmp.tile([128, T], F32, tag="rd")
                            nc.vector.reciprocal_approx_fast(out=rd, in_=dd)
                            t1 = tmp.tile([128, T], F32, tag="t1")
                            nc.vector.tensor_tensor(out=t1, in0=nps, in1=rd, op=ALU.mult)
                            nc.vector.tensor_tensor(out=YT[j], in0=t1, in1=eQ[j], op=ALU.mult)

                        # output projection + residual -> x1, plus LN2 stats
                        for i in range(4):
                            t = b * 4 + i
                            for h in range(2):
                                sl = slice(h * 512, (h + 1) * 512)
                                pps = ps_mm.tile([128, 512], F32, tag="mm")
                                for j in range(KD):
                                    nc.tensor.matmul(pps, lhsT=YT[j][:, i * 128:(i + 1) * 128],
                                                     rhs=wp_sb[j][:, sl], start=(j == 0),
                                                     stop=(not with_bias and j == KD - 1))
                                if with_bias:
                                    nc.tensor.matmul(pps, lhsT=ones1, rhs=brows[2][:, sl],
                                                     start=False, stop=True)
                                nc.vector.tensor_tensor(out=x1t[i][:, sl], in0=pps,
                                                        in1=xb[i][:, sl], op=ALU.add)
                            ln_stats(x1t[i], mv2, t, sip)
                            nc.sync.dma_start(out=x1d[t], in_=x1t[i])

                        # finalize this batch's LN2 rstd and the next batch's
                        # LN1 rstd (both pure-DVE, no ACT table involvement)
                        rsqrt_dve(rstd2, nmr2, mv2, b * 4, b * 4 + 4)
                        if b + 1 < NB:
                            xb = xb_next

            # ---------------- FFN sub-block ----------------
            # 512-token chunks; w1 streamed per f-tile, w2 resident,
            # gelu output G kept in SBUF; every matmul at N=512.
            with (
                tc.tile_pool(name="fw", bufs=1) as fw,
                tc.tile_pool(name="w1s", bufs=4) as w1s,
                tc.tile_pool(name="fb", bufs=1) as fb,
                tc.tile_pool(name="ftmp", bufs=3) as ftmp,
                tc.tile_pool(name="ps_o", bufs=3, space="PSUM") as ps_o,
                tc.tile_pool(name="ps_h1", bufs=2, space="PSUM") as ps_h1,
            ):
                w2_sb = [fw.tile([128, D], BF16, tag=f"w2_{f}", name=f"w2_{f}")
                         for f in range(KF)]

                for c in range(NB):  # 512-token chunks
                    x1c = [fb.tile([128, D], F32, tag=f"x1c{i}", bufs=2, name=f"x1c{i}")
                           for i in range(4)]
                    h2T = [fb.tile([128, T], BF16, tag=f"h2T{k}", bufs=2, name=f"h2T{k}")
                           for k in range(KD)]
                    G = [fb.tile([128, T], BF16, tag=f"G{f}", bufs=1, name=f"G{f}")
                         for f in range(KF)]
                    for i in range(4):
                        t = c * 4 + i
                        nc.sync.dma_start(out=x1c[i], in_=x1d[t])
                        h2 = ftmp.tile([128, D], BF16, tag="h2")
                        nc.scalar.activation(out=h2, in_=x1c[i], func=AF.Identity,
                                             bias=nmr2[:, t:t + 1], scale=rstd2[:, t:t + 1])
                        for k in range(KD):
                            tp = ps_t.tile([128, 128], BF16, tag="tp")
                            nc.tensor.transpose(tp, h2[:, k * 128:(k + 1) * 128], ident)
                            nc.vector.tensor_copy(out=h2T[k][:, i * 128:(i + 1) * 128], in_=tp)

                    # up-projection + exact gelu (w2 streams in behind the
                    # first chunk's w1 traffic)
                    for f in range(KF):
                        w1t = w1s.tile([128, D], BF16, tag="w1t")
                        nc.sync.dma_start(out=w1t, in_=w1f_ext[f, :, :])
                        if c == 0:
                            nc.sync.dma_start(out=w2_sb[f], in_=w2_ext[f * 128:(f + 1) * 128, :])
                        h1 = ps_h1.tile([128, T], F32, tag="h1")
                        for k in range(KD):
                            nc.tensor.matmul(h1, lhsT=w1t[:, k * 128:(k + 1) * 128],
                                             rhs=h2T[k], start=(k == 0), stop=(k == KD - 1))
                        nc.scalar.activation(out=G[f], in_=h1, func=AF.Gelu,
                                             bias=b1col[:, f:f + 1])

                    # down-projection + bias + residual
                    for i in range(4):
                        t = c * 4 + i
                        oc = ftmp.tile([128, D], F32, tag="oc")
                        for h in range(2):
                            sl = slice(h * 512, (h + 1) * 512)
                            ops = ps_o.tile([128, 512], F32, tag="o")
                            for f in range(KF):
                                nc.tensor.matmul(ops, lhsT=G[f][:, i * 128:(i + 1) * 128],
                                                 rhs=w2_sb[f][:, sl], start=(f == 0),
                                                 stop=(not with_bias and f == KF - 1))
                            if with_bias:
                                nc.tensor.matmul(ops, lhsT=ones1, rhs=brows[3][:, sl],
                                                 start=False, stop=True)
                            nc.vector.tensor_tensor(out=oc[:, sl], in0=ops,
                                                    in1=x1c[i][:, sl], op=ALU.add)
                        nc.sync.dma_start(out=out_ext[t * 128:(t + 1) * 128, :], in_=oc)

    nc.compile()
    return nc


_CACHE = {}


def _prep_inputs(x, gamma, beta, wq, bq, wk, bk, wv, bv, wp, bp, wbias, w1, b1, w2, b2):
    bf = ml_dtypes.bfloat16
    f32 = np.float32
    gamma = np.asarray(gamma, f32)
    beta = np.asarray(beta, f32)
    wq = np.asarray(wq, f32); wk = np.asarray(wk, f32)
    wv = np.asarray(wv, f32); wp = np.asarray(wp, f32)
    w1 = np.asarray(w1, f32); w2 = np.asarray(w2, f32)

    wq_m = (gamma[:, None] * wq).astype(bf)
    wk_m = (gamma[:, None] * wk).astype(bf)
    wv_m = (gamma[:, None] * wv).astype(bf)
    w1_m = gamma[:, None] * w1
    wp_m = wp.astype(bf)
    w2_m = w2.astype(bf)
    bq_m = beta @ wq + np.asarray(bq, f32)
    bk_m = beta @ wk + np.asarray(bk, f32)
    bv_m = beta @ wv + np.asarray(bv, f32)
    b1_m = beta @ w1 + np.asarray(b1, f32)
    bp_m = np.asarray(bp, f32)
    b2_m = np.asarray(b2, f32)
    wbT = np.exp(np.asarray(wbias, f32)[:T, :T]).T.astype(bf)

    # w1 tiled for per-f streaming: w1f[f, p, k*128+c] = w1_m[k*128+p, f*128+c]
    w1f = np.ascontiguousarray(
        w1_m.reshape(KD, 128, KF, 128).transpose(2, 1, 0, 3).reshape(KF, 128, D)
    ).astype(bf)

    brow = np.stack([bk_m, bv_m, bp_m, b2_m]).astype(bf)                  # [4, D]
    bqcol = np.ascontiguousarray(bq_m.reshape(KD, 128).T, f32)            # [128, KD]
    b1col = np.ascontiguousarray(b1_m.reshape(KF, 128).T, f32)            # [128, KF]

    with_bias = not (np.all(bk_m == 0) and np.all(bv_m == 0) and np.all(bp_m == 0)
                     and np.all(b2_m == 0))

    shared = dict(wq=wq_m, wk=wk_m, wv=wv_m, wp=wp_m, w1f=w1f, w2=w2_m,
                  wbT=np.ascontiguousarray(wbT), brow=brow, bqcol=bqcol, b1col=b1col)
    x = np.asarray(x, f32)
    in_maps = []
    for core in range(NCORES):
        shard = np.ascontiguousarray(x[core * NB:(core + 1) * NB].reshape(NT, D))
        in_maps.append(dict(shared, x=shard))
    return in_maps, with_bias


def kernel(**inputs) -> np.ndarray:
    in_maps, with_bias = _prep_inputs(**inputs)
    key = ("nc", with_bias)
    if key not in _CACHE:
        _CACHE[key] = build_nc(with_bias)
    nc = _CACHE[key]
    res = run_bass_kernel_spmd(nc, in_maps, core_ids=list(range(NCORES)))
    out = np.empty((B, T, D), np.float32)
    for core in range(NCORES):
        out[core * NB:(core + 1) * NB] = res.results[core]["out"].reshape(NB, T, D)
    return out


# revision 19
# speedup vs baseline: 1.0078x; 1.0078x over previous
"""AFT transformer block on 8 Trainium2 NeuronCores.

Data-parallel over batch: each core runs the full block for 4 of the 32
sequences (the AFT attention mixes only within a sequence, so no
collectives are needed).  Host side folds the shared LayerNorm affine
params into the GEMM weights, precomputes exp(wbias).T, and casts all
weights to bf16; the device kernel computes bf16 matmuls with f32 PSUM
accumulation and f32 element-wise math.

ScalarE table-set discipline: two table loads total (exp for attention,
gelu for the FFN).  LayerNorm rstd is computed entirely on the vector
engine (bit-trick seed + Newton rsqrt), sigmoid uses e/(1+e) with
e = exp(Q), and divisions use the fast DVE reciprocal approximation.
"""

import numpy as np
import ml_dtypes

import concourse.bass as bass
import concourse.mybir as mybir
import concourse.tile as tile
from concourse import bacc
from concourse.bass_utils import run_bass_kernel_spmd
from concourse.masks import make_identity

F32 = mybir.dt.float32
U32 = mybir.dt.uint32
BF16 = mybir.dt.bfloat16
AF = mybir.ActivationFunctionType
ALU = mybir.AluOpType

B, T, D, FF = 32, 512, 1024, 4096
NCORES = 8
NB = B // NCORES          # sequences per core (4)
NT = NB * T               # tokens per core (2048)
KD = D // 128             # 8
KF = FF // 128            # 32
TT = NT // 128            # 16 token tiles per core
EPS = 1e-5


def build_nc(with_bias: bool):
    nc = bacc.Bacc("TRN2", target_bir_lowering=False, debug=False, num_devices=NCORES)

    x_ext = nc.dram_tensor("x", [NT, D], F32, kind="ExternalInput").ap()
    wq_ext = nc.dram_tensor("wq", [D, D], BF16, kind="ExternalInput").ap()
    wk_ext = nc.dram_tensor("wk", [D, D], BF16, kind="ExternalInput").ap()
    wv_ext = nc.dram_tensor("wv", [D, D], BF16, kind="ExternalInput").ap()
    wp_ext = nc.dram_tensor("wp", [D, D], BF16, kind="ExternalInput").ap()
    # w1 pre-tiled on host: w1f[f, p, k*128+c] = w1[k*128+p, f*128+c]
    w1f_ext = nc.dram_tensor("w1f", [KF, 128, D], BF16, kind="ExternalInput").ap()
    w2_ext = nc.dram_tensor("w2", [FF, D], BF16, kind="ExternalInput").ap()
    wbT_ext = nc.dram_tensor("wbT", [T, T], BF16, kind="ExternalInput").ap()
    # rows: 0=bk', 1=bv', 2=bp', 3=b2' (rank-1 bias matmul operands)
    brow_ext = nc.dram_tensor("brow", [4, D], BF16, kind="ExternalInput").ap()
    bqcol_ext = nc.dram_tensor("bqcol", [128, KD], F32, kind="ExternalInput").ap()
    b1col_ext = nc.dram_tensor("b1col", [128, KF], F32, kind="ExternalInput").ap()
    out_ext = nc.dram_tensor("out", [NT, D], F32, kind="ExternalOutput").ap()

    with tile.TileContext(nc) as tc:
        with (
            tc.tile_pool(name="const", bufs=1) as constp,
            tc.tile_pool(name="dstats", bufs=1) as dstats,
            tc.tile_pool(name="ps_t", bufs=2, space="PSUM") as ps_t,
            tc.tile_pool(name="x1dp", bufs=1, space="DRAM") as x1dp,
        ):
            ident = constp.tile([128, 128], BF16, tag="ident")
            make_identity(nc, ident)
            ones1 = constp.tile([1, 128], BF16, tag="ones1")
            nc.vector.memset(ones1, 1.0)
            wbT_sb = constp.tile([128, 4, T], BF16, tag="wbT")
            brows = []
            for r in range(4):
                br = constp.tile([1, D], BF16, tag=f"brow{r}", name=f"brow{r}")
                nc.sync.dma_start(out=br, in_=brow_ext[r:r + 1, :])
                brows.append(br)
            bqcol = constp.tile([128, KD], F32, tag="bqcol")
            nc.sync.dma_start(out=bqcol, in_=bqcol_ext[:, :])
            b1col = constp.tile([128, KF], F32, tag="b1col")
            nc.sync.dma_start(out=b1col, in_=b1col_ext[:, :])

            # per-token-tile LayerNorm stats, [128, tt] (one column per tile)
            mv1 = dstats.tile([128, TT, 2], F32, tag="mv1")
            rstd1 = dstats.tile([128, TT], F32, tag="rstd1")
            nmr1 = dstats.tile([128, TT], F32, tag="nmr1")
            mv2 = dstats.tile([128, TT, 2], F32, tag="mv2")
            rstd2 = dstats.tile([128, TT], F32, tag="rstd2")
            nmr2 = dstats.tile([128, TT], F32, tag="nmr2")
            nsc = dstats.tile([128, 4, 2], F32, tag="nsc")  # rsqrt Newton scratch

            def rsqrt_dve(rstd_t, nmr_t, mv_t, lo, hi):
                """rstd = 1/sqrt(var+eps), nmr = -mean*rstd — DVE only."""
                n = hi - lo
                y = rstd_t[:, lo:hi]
                ve = nsc[:, 0:n, 0]
                t = nsc[:, 0:n, 1]
                nc.vector.tensor_scalar_add(out=ve, in0=mv_t[:, lo:hi, 1], scalar1=EPS)
                nc.vector.tensor_scalar(out=y.bitcast(U32), in0=ve.bitcast(U32),
                                        scalar1=1, scalar2=None,
                                        op0=ALU.logical_shift_right)
                nc.vector.tensor_scalar(out=y.bitcast(U32), in0=y.bitcast(U32),
                                        scalar1=0x5f3759df, scalar2=-1,
                                        op0=ALU.subtract, op1=ALU.mult)
                for _ in range(3):
                    nc.vector.tensor_tensor(out=t, in0=ve, in1=y, op=ALU.mult)
                    nc.vector.tensor_tensor(out=t, in0=t, in1=y, op=ALU.mult)
                    nc.vector.tensor_scalar(out=t, in0=t, scalar1=-0.5, scalar2=1.5,
                                            op0=ALU.mult, op1=ALU.add)
                    nc.vector.tensor_tensor(out=y, in0=y, in1=t, op=ALU.mult)
                nc.vector.tensor_tensor(out=nmr_t[:, lo:hi], in0=mv_t[:, lo:hi, 0],
                                        in1=y, op=ALU.mult)
                nc.vector.tensor_scalar_mul(out=nmr_t[:, lo:hi], in0=nmr_t[:, lo:hi],
                                            scalar1=-1.0)

            # DRAM scratch for the post-attention residual stream x1
            x1d = [x1dp.tile([128, D], F32, tag=f"x1d{t}", name=f"x1d{t}") for t in range(TT)]

            # ---------------- attention sub-block ----------------
            with (
                tc.tile_pool(name="aw", bufs=1) as aw,
                tc.tile_pool(name="xbp", bufs=2) as xbp,
                tc.tile_pool(name="ps_mm", bufs=6, space="PSUM") as ps_mm,
            ):
                # LN1 stats prologue.  Batch 0's activation tiles go first in
                # the DMA queue; stats for tiles 0-3 are finalized early so
                # batch 0 can start while the rest of the stats stream in.
                xb0 = [xbp.tile([128, D], F32, tag=f"xb{i}", name=f"xb{i}") for i in range(4)]
                for i in range(4):
                    nc.sync.dma_start(out=xb0[i], in_=x_ext[i * 128:(i + 1) * 128, :])

                def ln_stats(xt, mv_t, t, stats_pool):
                    st = stats_pool.tile([128, 2, 6], F32, tag="bnst", name="st")
                    nc.vector.bn_stats(out=st[:, 0, :], in_=xt[:, 0:512])
                    nc.vector.bn_stats(out=st[:, 1, :], in_=xt[:, 512:1024])
                    nc.vector.bn_aggr(out=mv_t[:, t, :], in_=st)

                wq_sb = [aw.tile([128, D], BF16, tag=f"wq{k}", name=f"wq{k}") for k in range(KD)]
                wk_sb = [aw.tile([128, D], BF16, tag=f"wk{k}", name=f"wk{k}") for k in range(KD)]
                wv_sb = [aw.tile([128, D], BF16, tag=f"wv{k}", name=f"wv{k}") for k in range(KD)]
                wp_sb = [aw.tile([128, D], BF16, tag=f"wp{k}", name=f"wp{k}") for k in range(KD)]
                for k in range(KD):
                    sl = slice(k * 128, (k + 1) * 128)
                    nc.sync.dma_start(out=wq_sb[k], in_=wq_ext[sl, :])
                for k in range(KD):
                    sl = slice(k * 128, (k + 1) * 128)
                    nc.sync.dma_start(out=wk_sb[k], in_=wk_ext[sl, :])
                    nc.sync.dma_start(out=wv_sb[k], in_=wv_ext[sl, :])

                for s in range(4):
                    nc.sync.dma_start(out=wbT_sb[:, s, :], in_=wbT_ext[s * 128:(s + 1) * 128, :])
                for k in range(KD):
                    sl = slice(k * 128, (k + 1) * 128)
                    nc.sync.dma_start(out=wp_sb[k], in_=wp_ext[sl, :])

                with (
                    tc.tile_pool(name="ab", bufs=1) as ab,
                    tc.tile_pool(name="tmp", bufs=2) as tmp,
                    tc.tile_pool(name="sip", bufs=3) as sip,
                ):
                    # batch-0 stats and rstd straight away (DVE only)
                    for t in range(4):
                        ln_stats(xb0[t], mv1, t, sip)
                    rsqrt_dve(rstd1, nmr1, mv1, 0, 4)

                    xb = xb0
                    for b in range(NB):
                        h0T = [ab.tile([128, T], BF16, tag=f"h0T{k}", bufs=2, name=f"h0T{k}")
                               for k in range(KD)]
                        # prefetch next batch's activations early in the queue
                        if b + 1 < NB:
                            xb_next = [xbp.tile([128, D], F32, tag=f"xb{i}", name=f"xb{i}")
                                       for i in range(4)]
                            for i in range(4):
                                t = (b + 1) * 4 + i
                                nc.sync.dma_start(out=xb_next[i],
                                                  in_=x_ext[t * 128:(t + 1) * 128, :])
                        eK = [ab.tile([128, D], BF16, tag=f"eK{i}", bufs=1, name=f"eK{i}")
                              for i in range(4)]
                        EV = [ab.tile([128, D], BF16, tag=f"EV{i}", bufs=1, name=f"EV{i}")
                              for i in range(4)]
                        eQ = [ab.tile([128, T], BF16, tag=f"eQ{j}", bufs=1, name=f"eQ{j}")
                              for j in range(KD)]
                        e1 = [ab.tile([128, T], BF16, tag=f"e1{j}", bufs=1, name=f"e1{j}")
                              for j in range(KD)]
                        YT = [ab.tile([128, T], BF16, tag=f"YT{j}", bufs=1, name=f"YT{j}")
                              for j in range(KD)]
                        x1t = [ab.tile([128, D], F32, tag=f"x1t{i}", bufs=1, name=f"x1t{i}")
                               for i in range(4)]

                        # normalize (pre-folded LN) + transpose to feature-major
                        for i in range(4):
                            t = b * 4 + i
                            h0 = tmp.tile([128, D], BF16, tag="h0", bufs=3)
                            nc.scalar.activation(out=h0, in_=xb[i], func=AF.Identity,
                                                 bias=nmr1[:, t:t + 1], scale=rstd1[:, t:t + 1])
                            for k in range(KD):
                                tp = ps_t.tile([128, 128], BF16, tag="tp")
                                nc.tensor.transpose(tp, h0[:, k * 128:(k + 1) * 128], ident)
                                nc.vector.tensor_copy(out=h0T[k][:, i * 128:(i + 1) * 128],
                                                      in_=tp)

                        # Q -> e = exp(Q + bq)  (sigmoid deferred into Y)
                        for j in range(KD):
                            qps = ps_mm.tile([128, T], F32, tag="mm")
                            for k in range(KD):
                                nc.tensor.matmul(qps, lhsT=wq_sb[k][:, j * 128:(j + 1) * 128],
                                                 rhs=h0T[k], start=(k == 0), stop=(k == KD - 1))
                            nc.scalar.activation(out=eQ[j], in_=qps, func=AF.Exp,
                                                 bias=bqcol[:, j:j + 1])
                            nc.gpsimd.tensor_scalar_add(out=e1[j], in0=eQ[j], scalar1=1.0)

                        # K, V (token-major) -> exp(K), exp(K)*V
                        for i in range(4):
                            for h in range(2):
                                sl = slice(h * 512, (h + 1) * 512)
                                kps = ps_mm.tile([128, 512], F32, tag="mm")
                                for k in range(KD):
                                    nc.tensor.matmul(kps, lhsT=h0T[k][:, i * 128:(i + 1) * 128],
                                                     rhs=wk_sb[k][:, sl], start=(k == 0),
                                                     stop=(not with_bias and k == KD - 1))
                                if with_bias:
                                    nc.tensor.matmul(kps, lhsT=ones1, rhs=brows[0][:, sl],
                                                     start=False, stop=True)
                                nc.scalar.activation(out=eK[i][:, sl], in_=kps, func=AF.Exp)
                                vps = ps_mm.tile([128, 512], F32, tag="mm")
                                for k in range(KD):
                                    nc.tensor.matmul(vps, lhsT=h0T[k][:, i * 128:(i + 1) * 128],
                                                     rhs=wv_sb[k][:, sl], start=(k == 0),
                                                     stop=(not with_bias and k == KD - 1))
                                if with_bias:
                                    nc.tensor.matmul(vps, lhsT=ones1, rhs=brows[1][:, sl],
                                                     start=False, stop=True)
                                nc.vector.tensor_tensor(out=EV[i][:, sl], in0=eK[i][:, sl],
                                                        in1=vps, op=ALU.mult)

                        # next batch's LN1 stats here in the DVE stream:
                        # late enough that xb_next's DMA has landed, early
                        # enough that rstd1(b+1) beats batch b+1's normalize
                        if b + 1 < NB:
                            for i in range(4):
                                ln_stats(xb_next[i], mv1, (b + 1) * 4 + i, sip)
                            rsqrt_dve(rstd1, nmr1, mv1, (b + 1) * 4, (b + 1) * 4 + 4)

                        # positional-bias matmuls (feature-major) + Y epilogue
                        # Y = num * e * 1/((1+e) * den)
                        for j in range(KD):
                            jsl = slice(j * 128, (j + 1) * 128)
                            dps = ps_mm.tile([128, T], F32, tag="mm")
                            for s in range(4):
                                nc.tensor.matmul(dps, lhsT=eK[s][:, jsl], rhs=wbT_sb[:, s, :],
                                                 start=(s == 0), stop=(s == 3))
                            nps = ps_mm.tile([128, T], F32, tag="mm")
                            for s in range(4):
                                nc.tensor.matmul(nps, lhsT=EV[s][:, jsl], rhs=wbT_sb[:, s, :],
                                                 start=(s == 0), stop=(s == 3))
                            dd = tmp.tile([128, T], F32, tag="dd")
                            nc.vector.tensor_tensor(out=dd, in0=dps, in1=e1[j], op=ALU.mult)
                            rd = tmp.tile([128, T], F32, tag="rd")
                            nc.vector.reciprocal_approx_fast(out=rd, in_=dd)
                            t1 = tmp.tile([128, T], F32, tag="t1")
                            nc.vector.tensor_tensor(out=t1, in0=nps, in1=rd, op=ALU.mult)
                            nc.vector.tensor_tensor(out=YT[j], in0=t1, in1=eQ[j], op=ALU.mult)

                        # output projection + residual -> x1, plus LN2 stats
                        for i in range(4):
                            t = b * 4 + i
                            for h in range(2):
                                sl = slice(h * 512, (h + 1) * 512)
                                pps = ps_mm.tile([128, 512], F32, tag="mm")
                                for j in range(KD):
                                    nc.tensor.matmul(pps, lhsT=YT[j][:, i * 128:(i + 1) * 128],
                                                     rhs=wp_sb[j][:, sl], start=(j == 0),
                                                     stop=(not with_bias and j == KD - 1))
                                if with_bias:
                                    nc.tensor.matmul(pps, lhsT=ones1, rhs=brows[2][:, sl],
                                                     start=False, stop=True)
                                nc.vector.tensor_tensor(out=x1t[i][:, sl], in0=pps,
                                                        in1=xb[i][:, sl], op=ALU.add)
                            ln_stats(x1t[i], mv2, t, sip)
                            nc.sync.dma_start(out=x1d[t], in_=x1t[i])

                        # finalize this batch's LN2 rstd and the next batch's
                        # LN1 rstd (both pure-DVE, no ACT table involvement)
                        rsqrt_dve(rstd2, nmr2, mv2, b * 4, b * 4 + 4)
                        if b + 1 < NB:
                            xb = xb_next

            # ---------------- FFN sub-block ----------------
            # 512-token chunks; w1 streamed per f-tile, w2 resident,
            # gelu output G kept in SBUF; every matmul at N=512.
            with (
                tc.tile_pool(name="fw", bufs=1) as fw,
                tc.tile_pool(name="w1s", bufs=4) as w1s,
                tc.tile_pool(name="fb", bufs=1) as fb,
                tc.tile_pool(name="ftmp", bufs=3) as ftmp,
                tc.tile_pool(name="ps_o", bufs=3, space="PSUM") as ps_o,
                tc.tile_pool(name="ps_h1", bufs=2, space="PSUM") as ps_h1,
            ):
                w2_sb = [fw.tile([128, D], BF16, tag=f"w2_{f}", name=f"w2_{f}")
                         for f in range(KF)]

                for c in range(NB):  # 512-token chunks
                    x1c = [fb.tile([128, D], F32, tag=f"x1c{i}", bufs=2, name=f"x1c{i}")
                           for i in range(4)]
                    h2T = [fb.tile([128, T], BF16, tag=f"h2T{k}", bufs=2, name=f"h2T{k}")
                           for k in range(KD)]
                    G = [fb.tile([128, T], BF16, tag=f"G{f}", bufs=1, name=f"G{f}")
                         for f in range(KF)]
                    for i in range(4):
                        t = c * 4 + i
                        nc.sync.dma_start(out=x1c[i], in_=x1d[t])
                        h2 = ftmp.tile([128, D], BF16, tag="h2")
                        nc.scalar.activation(out=h2, in_=x1c[i], func=AF.Identity,
                                             bias=nmr2[:, t:t + 1], scale=rstd2[:, t:t + 1])
                        for k in range(KD):
                            tp = ps_t.tile([128, 128], BF16, tag="tp")
                            nc.tensor.transpose(tp, h2[:, k * 128:(k + 1) * 128], ident)
                            nc.vector.tensor_copy(out=h2T[k][:, i * 128:(i + 1) * 128], in_=tp)

                    # up-projection + exact gelu (w2 streams in behind the
                    # first chunk's w1 traffic)
                    for f in range(KF):
                        w1t = w1s.tile([128, D], BF16, tag="w1t")
                        nc.sync.dma_start(out=w1t, in_=w1f_ext[f, :, :])
                        if c == 0:
                            nc.sync.dma_start(out=w2_sb[f], in_=w2_ext[f * 128:(f + 1) * 128, :])
                        h1 = ps_h1.tile([128, T], F32, tag="h1")
                        for k in range(KD):
                            nc.tensor.matmul(h1, lhsT=w1t[:, k * 128:(k + 1) * 128],
                                             rhs=h2T[k], start=(k == 0), stop=(k == KD - 1))
                        nc.scalar.activation(out=G[f], in_=h1, func=AF.Gelu,
                                             bias=b1col[:, f:f + 1])

                    # down-projection + bias + residual
                    for i in range(4):
                        t = c * 4 + i
                        oc = ftmp.tile([128, D], F32, tag="oc")
                        for h in range(2):
                            sl = slice(h * 512, (h + 1) * 512)
                            ops = ps_o.tile([128, 512], F32, tag="o")
                            for f in range(KF):
                                nc.tensor.matmul(ops, lhsT=G[f][:, i * 128:(i + 1) * 128],
                                                 rhs=w2_sb[f][:, sl], start=(f == 0),
                                                 stop=(not with_bias and f == KF - 1))
                            if with_bias:
                                nc.tensor.matmul(ops, lhsT=ones1, rhs=brows[3][:, sl],
                                                 start=False, stop=True)
                            nc.vector.tensor_tensor(out=oc[:, sl], in0=ops,
                                                    in1=x1c[i][:, sl], op=ALU.add)
                        nc.sync.dma_start(out=out_ext[t * 128:(t + 1) * 128, :], in_=oc)

    nc.compile()
    return nc


_CACHE = {}


def _prep_inputs(x, gamma, beta, wq, bq, wk, bk, wv, bv, wp, bp, wbias, w1, b1, w2, b2):
    bf = ml_dtypes.bfloat16
    f32 = np.float32
    gamma = np.asarray(gamma, f32)
    beta = np.asarray(beta, f32)
    wq = np.asarray(wq, f32); wk = np.asarray(wk, f32)
    wv = np.asarray(wv, f32); wp = np.asarray(wp, f32)
    w1 = np.asarray(w1, f32); w2 = np.asarray(w2, f32)

    wq_m = (gamma[:, None] * wq).astype(bf)
    wk_m = (gamma[:, None] * wk).astype(bf)
    wv_m = (gamma[:, None] * wv).astype(bf)
    w1_m = gamma[:, None] * w1
    wp_m = wp.astype(bf)
    w2_m = w2.astype(bf)
    bq_m = beta @ wq + np.asarray(bq, f32)
    bk_m = beta @ wk + np.asarray(bk, f32)
    bv_m = beta @ wv + np.asarray(bv, f32)
    b1_m = beta @ w1 + np.asarray(b1, f32)
    bp_m = np.asarray(bp, f32)
    b2_m = np.asarray(b2, f32)
    wbT = np.exp(np.asarray(wbias, f32)[:T, :T]).T.astype(bf)

    # w1 tiled for per-f streaming: w1f[f, p, k*128+c] = w1_m[k*128+p, f*128+c]
    w1f = np.ascontiguousarray(
        w1_m.reshape(KD, 128, KF, 128).transpose(2, 1, 0, 3).reshape(KF, 128, D)
    ).astype(bf)

    brow = np.stack([bk_m, bv_m, bp_m, b2_m]).astype(bf)                  # [4, D]
    bqcol = np.ascontiguousarray(bq_m.reshape(KD, 128).T, f32)            # [128, KD]
    b1col = np.ascontiguousarray(b1_m.reshape(KF, 128).T, f32)            # [128, KF]

    with_bias = not (np.all(bk_m == 0) and np.all(bv_m == 0) and np.all(bp_m == 0)
                     and np.all(b2_m == 0))

    shared = dict(wq=wq_m, wk=wk_m, wv=wv_m, wp=wp_m, w1f=w1f, w2=w2_m,
                  wbT=np.ascontiguousarray(wbT), brow=brow, bqcol=bqcol, b1col=b1col)
    x = np.asarray(x, f32)
    in_maps = []
    for core in range(NCORES):
        shard = np.ascontiguousarray(x[core * NB:(core + 1) * NB].reshape(NT, D))
        in_maps.append(dict(shared, x=shard))
    return in_maps, with_bias


def kernel(**inputs) -> np.ndarray:
    in_maps, with_bias = _prep_inputs(**inputs)
    key = ("nc", with_bias)
    if key not in _CACHE:
        _CACHE[key] = build_nc(with_bias)
    nc = _CACHE[key]
    res = run_bass_kernel_spmd(nc, in_maps, core_ids=list(range(NCORES)))
    out = np.empty((B, T, D), np.float32)
    for core in range(NCORES):
        out[core * NB:(core + 1) * NB] = res.results[core]["out"].reshape(NB, T, D)
    return out


# revision 24
# speedup vs baseline: 1.3244x; 1.3142x over previous
"""AFT transformer block on 8 Trainium2 NeuronCores.

Data-parallel over batch: each core runs the full block for 4 of the 32
sequences (the AFT attention mixes only within a sequence, so no
collectives are needed).  Host side folds the shared LayerNorm affine
params into the GEMM weights, precomputes exp(wbias).T, and casts all
weights to bf16; the device kernel computes bf16 matmuls with f32 PSUM
accumulation and f32 element-wise math.

ScalarE table-set discipline: two table loads total (exp for attention,
gelu for the FFN).  LayerNorm rstd is computed entirely on the vector
engine (bit-trick seed + Newton rsqrt), sigmoid uses e/(1+e) with
e = exp(Q), and divisions use the fast DVE reciprocal approximation.
"""

import numpy as np
import ml_dtypes

import concourse.bass as bass
import concourse.mybir as mybir
import concourse.tile as tile
from concourse import bacc
from concourse.bass_utils import run_bass_kernel_spmd
from concourse.masks import make_identity

F32 = mybir.dt.float32
U32 = mybir.dt.uint32
BF16 = mybir.dt.bfloat16
AF = mybir.ActivationFunctionType
ALU = mybir.AluOpType

B, T, D, FF = 32, 512, 1024, 4096
NCORES = 8
NB = B // NCORES          # sequences per core (4)
NT = NB * T               # tokens per core (2048)
KD = D // 128             # 8
KF = FF // 128            # 32
TT = NT // 128            # 16 token tiles per core
EPS = 1e-5


def build_nc(with_bias: bool, stats_pos: str = 'kv'):
    nc = bacc.Bacc("TRN2", target_bir_lowering=False, debug=False, num_devices=NCORES)

    x_ext = nc.dram_tensor("x", [NT, D], F32, kind="ExternalInput").ap()
    wq_ext = nc.dram_tensor("wq", [D, D], BF16, kind="ExternalInput").ap()
    wk_ext = nc.dram_tensor("wk", [D, D], BF16, kind="ExternalInput").ap()
    wv_ext = nc.dram_tensor("wv", [D, D], BF16, kind="ExternalInput").ap()
    wp_ext = nc.dram_tensor("wp", [D, D], BF16, kind="ExternalInput").ap()
    # w1 pre-tiled on host: w1f[f, p, k*128+c] = w1[k*128+p, f*128+c]
    w1f_ext = nc.dram_tensor("w1f", [KF, 128, D], BF16, kind="ExternalInput").ap()
    w2_ext = nc.dram_tensor("w2", [FF, D], BF16, kind="ExternalInput").ap()
    wbT_ext = nc.dram_tensor("wbT", [T, T], BF16, kind="ExternalInput").ap()
    # rows: 0=bk', 1=bv', 2=bp', 3=b2' (rank-1 bias matmul operands)
    brow_ext = nc.dram_tensor("brow", [4, D], BF16, kind="ExternalInput").ap()
    bqcol_ext = nc.dram_tensor("bqcol", [128, KD], F32, kind="ExternalInput").ap()
    b1col_ext = nc.dram_tensor("b1col", [128, KF], F32, kind="ExternalInput").ap()
    out_ext = nc.dram_tensor("out", [NT, D], F32, kind="ExternalOutput").ap()

    with tile.TileContext(nc) as tc:
        with (
            tc.tile_pool(name="const", bufs=1) as constp,
            tc.tile_pool(name="dstats", bufs=1) as dstats,
            tc.tile_pool(name="ps_t", bufs=2, space="PSUM") as ps_t,
            tc.tile_pool(name="x1dp", bufs=1, space="DRAM") as x1dp,
        ):
            ident = constp.tile([128, 128], BF16, tag="ident")
            make_identity(nc, ident)
            ones1 = constp.tile([1, 128], BF16, tag="ones1")
            nc.vector.memset(ones1, 1.0)
            wbT_sb = constp.tile([128, 4, T], BF16, tag="wbT")
            brows = []
            for r in range(4):
                br = constp.tile([1, D], BF16, tag=f"brow{r}", name=f"brow{r}")
                nc.sync.dma_start(out=br, in_=brow_ext[r:r + 1, :])
                brows.append(br)
            bqcol = constp.tile([128, KD], F32, tag="bqcol")
            nc.sync.dma_start(out=bqcol, in_=bqcol_ext[:, :])
            b1col = constp.tile([128, KF], F32, tag="b1col")
            nc.sync.dma_start(out=b1col, in_=b1col_ext[:, :])

            # per-token-tile LayerNorm stats, [128, tt] (one column per tile)
            mean1 = dstats.tile([128, TT], F32, tag="mean1")
            var1 = dstats.tile([128, TT], F32, tag="var1")
            rstd1 = dstats.tile([128, TT], F32, tag="rstd1")
            nmr1 = dstats.tile([128, TT], F32, tag="nmr1")
            mean2 = dstats.tile([128, TT], F32, tag="mean2")
            var2 = dstats.tile([128, TT], F32, tag="var2")
            rstd2 = dstats.tile([128, TT], F32, tag="rstd2")
            nmr2 = dstats.tile([128, TT], F32, tag="nmr2")
            nve = dstats.tile([128, 4], F32, tag="nve")   # rsqrt Newton scratch
            nt = dstats.tile([128, 4], F32, tag="nt")

            def rsqrt_dve(rstd_t, nmr_t, mean_t, var_t, lo, hi):
                """rstd = 1/sqrt(var+eps), nmr = -mean*rstd — DVE only."""
                n = hi - lo
                y = rstd_t[:, lo:hi]
                ve = nve[:, 0:n]
                t = nt[:, 0:n]
                nc.vector.tensor_scalar_add(out=ve, in0=var_t[:, lo:hi], scalar1=EPS)
                nc.vector.tensor_scalar(out=y.bitcast(U32), in0=ve.bitcast(U32),
                                        scalar1=1, scalar2=None,
                                        op0=ALU.logical_shift_right)
                nc.vector.tensor_scalar(out=y.bitcast(U32), in0=y.bitcast(U32),
                                        scalar1=0x5f3759df, scalar2=-1,
                                        op0=ALU.subtract, op1=ALU.mult)
                for _ in range(3):
                    nc.vector.tensor_tensor(out=t, in0=ve, in1=y, op=ALU.mult)
                    nc.vector.tensor_tensor(out=t, in0=t, in1=y, op=ALU.mult)
                    nc.vector.tensor_scalar(out=t, in0=t, scalar1=-0.5, scalar2=1.5,
                                            op0=ALU.mult, op1=ALU.add)
                    nc.vector.tensor_tensor(out=y, in0=y, in1=t, op=ALU.mult)
                nc.vector.tensor_tensor(out=nmr_t[:, lo:hi], in0=mean_t[:, lo:hi],
                                        in1=y, op=ALU.mult)
                nc.vector.tensor_scalar_mul(out=nmr_t[:, lo:hi], in0=nmr_t[:, lo:hi],
                                            scalar1=-1.0)

            # DRAM scratch for the post-attention residual stream x1
            x1d = [x1dp.tile([128, D], F32, tag=f"x1d{t}", name=f"x1d{t}") for t in range(TT)]

            # ---------------- attention sub-block ----------------
            with (
                tc.tile_pool(name="aw", bufs=1) as aw,
                tc.tile_pool(name="xbp", bufs=2) as xbp,
                tc.tile_pool(name="ps_mm", bufs=6, space="PSUM") as ps_mm,
            ):
                # LN1 stats prologue.  Batch 0's activation tiles go first in
                # the DMA queue; stats for tiles 0-3 are finalized early so
                # batch 0 can start while the rest of the stats stream in.
                xb0 = [xbp.tile([128, D], F32, tag=f"xb{i}", name=f"xb{i}") for i in range(4)]
                for i in range(4):
                    nc.sync.dma_start(out=xb0[i], in_=x_ext[i * 128:(i + 1) * 128, :])

                def ln_stats(xt, mean_t, var_t, t, stats_pool):
                    st = stats_pool.tile([128, 2, 6], F32, tag="bnst", name="st")
                    nc.vector.bn_stats(out=st[:, 0, :], in_=xt[:, 0:512])
                    nc.vector.bn_stats(out=st[:, 1, :], in_=xt[:, 512:1024])
                    mvt = stats_pool.tile([128, 2], F32, tag="mvt", name="mvt")
                    nc.vector.bn_aggr(out=mvt, in_=st)
                    nc.gpsimd.tensor_copy(out=mean_t[:, t:t + 1], in_=mvt[:, 0:1])
                    nc.gpsimd.tensor_copy(out=var_t[:, t:t + 1], in_=mvt[:, 1:2])

                wq_sb = [aw.tile([128, D], BF16, tag=f"wq{k}", name=f"wq{k}") for k in range(KD)]
                wk_sb = [aw.tile([128, D], BF16, tag=f"wk{k}", name=f"wk{k}") for k in range(KD)]
                wv_sb = [aw.tile([128, D], BF16, tag=f"wv{k}", name=f"wv{k}") for k in range(KD)]
                wp_sb = [aw.tile([128, D], BF16, tag=f"wp{k}", name=f"wp{k}") for k in range(KD)]
                for k in range(KD):
                    sl = slice(k * 128, (k + 1) * 128)
                    nc.sync.dma_start(out=wq_sb[k], in_=wq_ext[sl, :])
                for k in range(KD):
                    sl = slice(k * 128, (k + 1) * 128)
                    nc.sync.dma_start(out=wk_sb[k], in_=wk_ext[sl, :])
                    nc.sync.dma_start(out=wv_sb[k], in_=wv_ext[sl, :])

                for s in range(4):
                    nc.sync.dma_start(out=wbT_sb[:, s, :], in_=wbT_ext[s * 128:(s + 1) * 128, :])
                for k in range(KD):
                    sl = slice(k * 128, (k + 1) * 128)
                    nc.sync.dma_start(out=wp_sb[k], in_=wp_ext[sl, :])

                with (
                    tc.tile_pool(name="ab", bufs=1) as ab,
                    tc.tile_pool(name="tmp", bufs=2) as tmp,
                    tc.tile_pool(name="sip", bufs=3) as sip,
                ):
                    # batch-0 stats and rstd straight away (DVE only);
                    # tile 0 finalized alone so its normalize can start early
                    ln_stats(xb0[0], mean1, var1, 0, sip)
                    rsqrt_dve(rstd1, nmr1, mean1, var1, 0, 1)
                    for t in range(1, 4):
                        ln_stats(xb0[t], mean1, var1, t, sip)
                    rsqrt_dve(rstd1, nmr1, mean1, var1, 1, 4)

                    xb = xb0
                    for b in range(NB):
                        h0T = [ab.tile([128, T], BF16, tag=f"h0T{k}", bufs=2, name=f"h0T{k}")
                               for k in range(KD)]
                        # prefetch next batch's activations early in the queue
                        if b + 1 < NB:
                            xb_next = [xbp.tile([128, D], F32, tag=f"xb{i}", name=f"xb{i}")
                                       for i in range(4)]
                            for i in range(4):
                                t = (b + 1) * 4 + i
                                nc.sync.dma_start(out=xb_next[i],
                                                  in_=x_ext[t * 128:(t + 1) * 128, :])
                        eK = [ab.tile([128, D], BF16, tag=f"eK{i}", bufs=1, name=f"eK{i}")
                              for i in range(4)]
                        EV = [ab.tile([128, D], BF16, tag=f"EV{i}", bufs=1, name=f"EV{i}")
                              for i in range(4)]
                        eQ = [ab.tile([128, T], BF16, tag=f"eQ{j}", bufs=1, name=f"eQ{j}")
                              for j in range(KD)]
                        e1 = [ab.tile([128, T], BF16, tag=f"e1{j}", bufs=1, name=f"e1{j}")
                              for j in range(KD)]
                        YT = [ab.tile([128, T], BF16, tag=f"YT{j}", bufs=1, name=f"YT{j}")
                              for j in range(KD)]
                        x1t = [ab.tile([128, D], F32, tag=f"x1t{i}", bufs=1, name=f"x1t{i}")
                               for i in range(4)]

                        # normalize (pre-folded LN) + transpose to feature-major
                        for i in range(4):
                            t = b * 4 + i
                            h0 = tmp.tile([128, D], BF16, tag="h0", bufs=3)
                            nc.scalar.activation(out=h0, in_=xb[i], func=AF.Identity,
                                                 bias=nmr1[:, t:t + 1], scale=rstd1[:, t:t + 1])
                            for k in range(KD):
                                tp = ps_t.tile([128, 128], BF16, tag="tp")
                                nc.tensor.transpose(tp, h0[:, k * 128:(k + 1) * 128], ident)
                                nc.vector.tensor_copy(out=h0T[k][:, i * 128:(i + 1) * 128],
                                                      in_=tp)

                        # Q -> e = exp(Q + bq)  (sigmoid deferred into Y)
                        for j in range(KD):
                            qps = ps_mm.tile([128, T], F32, tag="mm")
                            for k in range(KD):
                                nc.tensor.matmul(qps, lhsT=wq_sb[k][:, j * 128:(j + 1) * 128],
                                                 rhs=h0T[k], start=(k == 0), stop=(k == KD - 1))
                            nc.scalar.activation(out=eQ[j], in_=qps, func=AF.Exp,
                                                 bias=bqcol[:, j:j + 1])
                            nc.vector.tensor_scalar_add(out=e1[j], in0=eQ[j], scalar1=1.0)

                        # K, V (token-major) -> exp(K), exp(K)*V
                        for i in range(4):
                            for h in range(2):
                                sl = slice(h * 512, (h + 1) * 512)
                                kps = ps_mm.tile([128, 512], F32, tag="mm")
                                for k in range(KD):
                                    nc.tensor.matmul(kps, lhsT=h0T[k][:, i * 128:(i + 1) * 128],
                                                     rhs=wk_sb[k][:, sl], start=(k == 0),
                                                     stop=(not with_bias and k == KD - 1))
                                if with_bias:
                                    nc.tensor.matmul(kps, lhsT=ones1, rhs=brows[0][:, sl],
                                                     start=False, stop=True)
                                nc.scalar.activation(out=eK[i][:, sl], in_=kps, func=AF.Exp)
                                vps = ps_mm.tile([128, 512], F32, tag="mm")
                                for k in range(KD):
                                    nc.tensor.matmul(vps, lhsT=h0T[k][:, i * 128:(i + 1) * 128],
                                                     rhs=wv_sb[k][:, sl], start=(k == 0),
                                                     stop=(not with_bias and k == KD - 1))
                                if with_bias:
                                    nc.tensor.matmul(vps, lhsT=ones1, rhs=brows[1][:, sl],
                                                     start=False, stop=True)
                                nc.vector.tensor_tensor(out=EV[i][:, sl], in0=eK[i][:, sl],
                                                        in1=vps, op=ALU.mult)
                            # interleave next batch's LN1 stats into the idle
                            # stretches of the K/V section's DVE stream
                            if stats_pos == 'kv' and b + 1 < NB:
                                ln_stats(xb_next[i], mean1, var1, (b + 1) * 4 + i, sip)
                        if stats_pos == 'kv' and b + 1 < NB:
                            rsqrt_dve(rstd1, nmr1, mean1, var1, (b + 1) * 4, (b + 1) * 4 + 4)

                        if stats_pos == 'mid' and b + 1 < NB:
                            for i in range(4):
                                ln_stats(xb_next[i], mean1, var1, (b + 1) * 4 + i, sip)
                            rsqrt_dve(rstd1, nmr1, mean1, var1, (b + 1) * 4, (b + 1) * 4 + 4)

                        # positional-bias matmuls (feature-major) + Y epilogue
                        # Y = num * e * 1/((1+e) * den)
                        for j in range(KD):
                            jsl = slice(j * 128, (j + 1) * 128)
                            dps = ps_mm.tile([128, T], F32, tag="mm")
                            for s in range(4):
                                nc.tensor.matmul(dps, lhsT=eK[s][:, jsl], rhs=wbT_sb[:, s, :],
                                                 start=(s == 0), stop=(s == 3))
                            nps = ps_mm.tile([128, T], F32, tag="mm")
                            for s in range(4):
                                nc.tensor.matmul(nps, lhsT=EV[s][:, jsl], rhs=wbT_sb[:, s, :],
                                                 start=(s == 0), stop=(s == 3))
                            dd = tmp.tile([128, T], F32, tag="dd")
                            nc.vector.tensor_tensor(out=dd, in0=dps, in1=e1[j], op=ALU.mult)
                            rd = tmp.tile([128, T], F32, tag="rd")
                            nc.vector.reciprocal_approx_fast(out=rd, in_=dd)
                            t1 = tmp.tile([128, T], F32, tag="t1")
                            nc.vector.tensor_tensor(out=t1, in0=nps, in1=rd, op=ALU.mult)
                            nc.vector.tensor_tensor(out=YT[j], in0=t1, in1=eQ[j], op=ALU.mult)

                        # output projection + residual -> x1, plus LN2 stats
                        for i in range(4):
                            t = b * 4 + i
                            for h in range(2):
                                sl = slice(h * 512, (h + 1) * 512)
                                pps = ps_mm.tile([128, 512], F32, tag="mm")
                                for j in range(KD):
                                    nc.tensor.matmul(pps, lhsT=YT[j][:, i * 128:(i + 1) * 128],
                                                     rhs=wp_sb[j][:, sl], start=(j == 0),
                                                     stop=(not with_bias and j == KD - 1))
                                if with_bias:
                                    nc.tensor.matmul(pps, lhsT=ones1, rhs=brows[2][:, sl],
                                                     start=False, stop=True)
                                nc.vector.tensor_tensor(out=x1t[i][:, sl], in0=pps,
                                                        in1=xb[i][:, sl], op=ALU.add)
                            ln_stats(x1t[i], mean2, var2, t, sip)
                            nc.sync.dma_start(out=x1d[t], in_=x1t[i])

                        # finalize this batch's LN2 rstd and the next batch's
                        # LN1 rstd (both pure-DVE, no ACT table involvement)
                        rsqrt_dve(rstd2, nmr2, mean2, var2, b * 4, b * 4 + 4)
                        if stats_pos == 'tail' and b + 1 < NB:
                            for i in range(4):
                                ln_stats(xb_next[i], mean1, var1, (b + 1) * 4 + i, sip)
                            rsqrt_dve(rstd1, nmr1, mean1, var1, (b + 1) * 4, (b + 1) * 4 + 4)
                        if b + 1 < NB:
                            xb = xb_next

            # ---------------- FFN sub-block ----------------
            # 512-token chunks; w1 streamed per f-tile, w2 resident,
            # gelu output G kept in SBUF; every matmul at N=512.
            with (
                tc.tile_pool(name="fw", bufs=1) as fw,
                tc.tile_pool(name="w1s", bufs=4) as w1s,
                tc.tile_pool(name="fb", bufs=1) as fb,
                tc.tile_pool(name="ftmp", bufs=3) as ftmp,
                tc.tile_pool(name="ps_o", bufs=3, space="PSUM") as ps_o,
                tc.tile_pool(name="ps_h1", bufs=2, space="PSUM") as ps_h1,
            ):
                w2_sb = [fw.tile([128, D], BF16, tag=f"w2_{f}", name=f"w2_{f}")
                         for f in range(KF)]

                for c in range(NB):  # 512-token chunks
                    x1c = [fb.tile([128, D], F32, tag=f"x1c{i}", bufs=2, name=f"x1c{i}")
                           for i in range(4)]
                    h2T = [fb.tile([128, T], BF16, tag=f"h2T{k}", bufs=2, name=f"h2T{k}")
                           for k in range(KD)]
                    G = [fb.tile([128, T], BF16, tag=f"G{f}", bufs=1, name=f"G{f}")
                         for f in range(KF)]
                    for i in range(4):
                        t = c * 4 + i
                        nc.sync.dma_start(out=x1c[i], in_=x1d[t])
                        h2 = ftmp.tile([128, D], BF16, tag="h2")
                        nc.scalar.activation(out=h2, in_=x1c[i], func=AF.Identity,
                                             bias=nmr2[:, t:t + 1], scale=rstd2[:, t:t + 1])
                        for k in range(KD):
                            tp = ps_t.tile([128, 128], BF16, tag="tp")
                            nc.tensor.transpose(tp, h2[:, k * 128:(k + 1) * 128], ident)
                            nc.vector.tensor_copy(out=h2T[k][:, i * 128:(i + 1) * 128], in_=tp)

                    # up-projection + exact gelu (w2 streams in behind the
                    # first chunk's w1 traffic)
                    for f in range(KF):
                        w1t = w1s.tile([128, D], BF16, tag="w1t")
                        nc.sync.dma_start(out=w1t, in_=w1f_ext[f, :, :])
                        if c == 0:
                            nc.sync.dma_start(out=w2_sb[f], in_=w2_ext[f * 128:(f + 1) * 128, :])
                        h1 = ps_h1.tile([128, T], F32, tag="h1")
                        for k in range(KD):
                            nc.tensor.matmul(h1, lhsT=w1t[:, k * 128:(k + 1) * 128],
                                             rhs=h2T[k], start=(k == 0), stop=(k == KD - 1))
                        nc.scalar.activation(out=G[f], in_=h1, func=AF.Gelu,
                                             bias=b1col[:, f:f + 1])

                    # down-projection + bias + residual
                    for i in range(4):
                        t = c * 4 + i
                        oc = ftmp.tile([128, D], F32, tag="oc")
                        for h in range(2):
                            sl = slice(h * 512, (h + 1) * 512)
                            ops = ps_o.tile([128, 512], F32, tag="o")
                            for f in range(KF):
                                nc.tensor.matmul(ops, lhsT=G[f][:, i * 128:(i + 1) * 128],
                                                 rhs=w2_sb[f][:, sl], start=(f == 0),
                                                 stop=(not with_bias and f == KF - 1))
                            if with_bias:
                                nc.tensor.matmul(ops, lhsT=ones1, rhs=brows[3][:, sl],
                                                 start=False, stop=True)
                            nc.vector.tensor_tensor(out=oc[:, sl], in0=ops,
                                                    in1=x1c[i][:, sl], op=ALU.add)
                        nc.sync.dma_start(out=out_ext[t * 128:(t + 1) * 128, :], in_=oc)

    nc.compile()
    return nc


_CACHE = {}


def _prep_inputs(x, gamma, beta, wq, bq, wk, bk, wv, bv, wp, bp, wbias, w1, b1, w2, b2):
    bf = ml_dtypes.bfloat16
    f32 = np.float32
    gamma = np.asarray(gamma, f32)
    beta = np.asarray(beta, f32)
    wq = np.asarray(wq, f32); wk = np.asarray(wk, f32)
    wv = np.asarray(wv, f32); wp = np.asarray(wp, f32)
    w1 = np.asarray(w1, f32); w2 = np.asarray(w2, f32)

    wq_m = (gamma[:, None] * wq).astype(bf)
    wk_m = (gamma[:, None] * wk).astype(bf)
    wv_m = (gamma[:, None] * wv).astype(bf)
    w1_m = gamma[:, None] * w1
    wp_m = wp.astype(bf)
    w2_m = w2.astype(bf)
    bq_m = beta @ wq + np.asarray(bq, f32)
    bk_m = beta @ wk + np.asarray(bk, f32)
    bv_m = beta @ wv + np.asarray(bv, f32)
    b1_m = beta @ w1 + np.asarray(b1, f32)
    bp_m = np.asarray(bp, f32)
    b2_m = np.asarray(b2, f32)
    wbT = np.exp(np.asarray(wbias, f32)[:T, :T]).T.astype(bf)

    # w1 tiled for per-f streaming: w1f[f, p, k*128+c] = w1_m[k*128+p, f*128+c]
    w1f = np.ascontiguousarray(
        w1_m.reshape(KD, 128, KF, 128).transpose(2, 1, 0, 3).reshape(KF, 128, D)
    ).astype(bf)

    brow = np.stack([bk_m, bv_m, bp_m, b2_m]).astype(bf)                  # [4, D]
    bqcol = np.ascontiguousarray(bq_m.reshape(KD, 128).T, f32)            # [128, KD]
    b1col = np.ascontiguousarray(b1_m.reshape(KF, 128).T, f32)            # [128, KF]

    with_bias = not (np.all(bk_m == 0) and np.all(bv_m == 0) and np.all(bp_m == 0)
                     and np.all(b2_m == 0))

    shared = dict(wq=wq_m, wk=wk_m, wv=wv_m, wp=wp_m, w1f=w1f, w2=w2_m,
                  wbT=np.ascontiguousarray(wbT), brow=brow, bqcol=bqcol, b1col=b1col)
    x = np.asarray(x, f32)
    in_maps = []
    for core in range(NCORES):
        shard = np.ascontiguousarray(x[core * NB:(core + 1) * NB].reshape(NT, D))
        in_maps.append(dict(shared, x=shard))
    return in_maps, with_bias


def kernel(**inputs) -> np.ndarray:
    in_maps, with_bias = _prep_inputs(**inputs)
    key = ("nc", with_bias)
    if key not in _CACHE:
        _CACHE[key] = build_nc(with_bias)
    nc = _CACHE[key]
    res = run_bass_kernel_spmd(nc, in_maps, core_ids=list(range(NCORES)))
    out = np.empty((B, T, D), np.float32)
    for core in range(NCORES):
        out[core * NB:(core + 1) * NB] = res.results[core]["out"].reshape(NB, T, D)
    return out
